# revision 37
# baseline (speedup 1.0000x reference)
"""Trainium2 Bass kernel: single-head attention with QKV+output projections.

Reference math (B=4, S=4096, D=64):
    Q = q@Wq.T+bq; K = k@Wk.T+bk; V = v@Wv.T+bv
    s = (Q @ K.T) / 8, masked -inf where i > j  (query i attends keys j >= i)
    out = softmax(s) @ V @ Wp.T + bp

Sharding (8 cores): core c -> batch b = c//2, parity h = c%2; 16 local
query slots of 128 rows (global tiles t = 2s+h), all 4096 keys.

Weight folding (host, O(D^3) only):
    S[i,j] = g_i . k_j + c_i ; the per-query c_i cancels in softmax, so
        g = M64 @ [q;1],  M64 = [A^T | Wk^T bq]  (64x65),  A = Wq^T Wk
        and scores use a pure 64-row k stationary (fast weight load).
    o_i = (sum_j p_ij vh_j) / l_i,  vh_j = [Wpv v_j + c; 1],
        Wpv = Wp Wv,  c = Wp bv + bp  (bp folds in since sum_j p_ij/l_i = 1;
        the ones column accumulates l alongside the output).

Device, per key block kb = 31..0 (descending; slot s finalizes at kb=2s):
    scores:  p_strip[128k, Nq(kb)] = kta_blk[64,128]^T @ G[64,Nq]
    exp:     SPLIT across two engines so the strip double-buffer drains
             ~2x faster and the PE stays dense:
               even units -> ACT exp (scale=1/8, bf16 out)
               odd units  -> DVE Schraudolph: i16=round(23.083*s+16248.59)
                 through an int16-bitcast AP; the bits ARE bf16 exp(s/8)
                 to ~2% rms (softmax renormalizes; l stays consistent
                 because it is summed from the same P values).
    mask:    GpSimd tril/zeros multiply on the boundary slot (sem_gp),
             keeping DVE free for exp work.
    PV flip: p_out[128q, 65] += P_tile[128,128]^T @ vhat_blk[128,65];
             the moving operand is the SMALL side, and ldweights pipeline
             under it (measured 54ns/slot = 65cyc at the fixed 1.2GHz).
    vhat     on-PE in batches (65-col matmuls from vta x folded N) through
             a 1-bank PSUM scratch; DVE copies it out.
    output:  NO on-device softmax divide - the raw accumulator (with l in
             column 64 of each slot) is copied to SBUF as bf16 in two
             waves (slots 8-15 after PV(16), 0-7 at the end) and DMA'd;
             the host divides by l. Kills the Ln/Exp/mult finale chain.

Prologue: DMA issue is spread over Sync+ACT queues; kta streams in three
chunks (kb 28-31 first) and qta in halves so G and the first score
blocks start as soon as their slice lands; a 1-elem exp right after mt
lands preloads the ACT table off the critical path.

HW notes: PE is pinned at the 1.2GHz p-state here (no HAM unthrottle,
never ramps to 2.4); matmul cost ~= moving columns x 0.833ns with
ldweights hidden, so the floor is ~47us of moving columns (scores 34816
+ PV 65x272 + vhat 2080 + G 2048) plus sem/bubble overhead. fp8
anywhere (even stationary-only) measurably slows the whole chip - keep
everything bf16. Measured ~77us (baseline 79us; 144us transposed
layout). Engine busy: PE ~64us (bound), ACT ~38, DVE ~35, GpSimd ~25.
"""

import numpy as np

import concourse.bass as bass
import concourse.mybir as mybir
from concourse.bass_utils import run_bass_kernel_spmd

B, S, D = 4, 4096, 64
NSLOT = 16
NKB = 32
QL = NSLOT * 128  # 2048
VST = 80  # vhat sbuf stride (65 used)

FP = mybir.dt.float32
BF = mybir.dt.bfloat16
I16 = mybir.dt.int16

# Schraudolph bf16-exp constants: bits(bf16 exp(s/8)) ~= 16*log2(e)*s +
# 128*(127 - sigma), sigma=0.0579 zero-mean
SCH_A = 23.083120654223414
SCH_B = 16248.5889

# vhat production batches (ascending kb within each so the batched DVE
# copy-out has positive strides); batch j>=1 is emitted on PE after the
# scores of VH_EMIT_AT[j], its copy goes after the DVE exp of VH_COPY_AT[j]
VH_BATCHES = [list(range(25, 32)), list(range(18, 25)),
              list(range(11, 18)), list(range(4, 11)),
              list(range(0, 4))]
VH_EMIT_AT = {1: 29, 2: 27, 3: 25, 4: 23}
VH_COPY_AT = {0: 1, 1: 3, 2: 7, 3: 11, 4: 15}  # after DVE exp of unit u


def dve_unit(u):
    return u % 2 == 1


def po_off(s):
    """p_out packing: 7 tiles per 512-f32 bank, stride 65."""
    return 512 * (s // 7) + 65 * (s % 7)


def nslots(kb):
    return min(kb // 2 + 1, NSLOT)


def bnd_unit(kb):
    """Unit index whose strip contains kb's boundary slot."""
    return 2 * (31 - kb) + 1 if kb >= 16 else 32 + (15 - kb)


def chunks(lo, hi, step):
    out = []
    c = lo
    while c < hi:
        out.append((c, min(c + step, hi)))
        c = out[-1][1]
    return out


def make_units():
    """Descending blocks, strip-chunked at 1024. Returns [(kb, lo, hi)]."""
    units = []
    for kb in range(NKB - 1, -1, -1):
        for lo, hi in chunks(0, 128 * nslots(kb), 1024):
            units.append((kb, lo, hi))
    return units


def tile_groups(s_lo, s_hi):
    """Split slot range at 7-tile PSUM bank-group boundaries."""
    out = []
    a = s_lo
    while a < s_hi:
        b = min(s_hi, (a // 7 + 1) * 7)
        out.append((a, b))
        a = b
    return out


def make_plan():
    units = make_units()
    last_unit = {}
    for u, (kb, lo, hi) in enumerate(units):
        last_unit[kb] = u

    p = {"units": units, "last_unit": last_unit}

    # --- PE stream ---
    pe = 0
    pe += 2  # G emitted as 4 chunks of 512 (PSUM bank limit)
    pe += 2
    p["pe_after_g"] = {0: 2, 1: 4}
    pe_vhb = {}
    p["pe_after_sunit"] = {}
    p["pe_after_pv"] = {}
    for u, (kb, lo, hi) in enumerate(units):
        pe += len(chunks(lo, hi, 512))
        p["pe_after_sunit"][u] = pe
        if u == last_unit[kb]:
            if kb == NKB - 1:
                pe += len(VH_BATCHES[0])
                pe_vhb[0] = pe
            for j, at in VH_EMIT_AT.items():
                if kb == at:
                    pe += len(VH_BATCHES[j])
                    pe_vhb[j] = pe
            if kb < NKB - 2:
                pe += nslots(kb + 2)
                p["pe_after_pv"][kb + 2] = pe
    pe += nslots(1)
    p["pe_after_pv"][1] = pe
    pe += nslots(0)
    p["pe_after_pv"][0] = pe
    p["pe_after_vhb"] = pe_vhb
    p["pe_total"] = pe

    # --- ACT stream: preload + even-unit exps (no finale: host divides) ---
    act = 1
    p["act_after_unit"] = {}
    for u, (kb, lo, hi) in enumerate(units):
        if dve_unit(u):
            continue
        act += 1
        p["act_after_unit"][u] = act
    p["act_total"] = act

    # --- DVE stream: casts + odd-unit exps + vh copies + finale mults ---
    dve = 0
    dve += 2  # G casts
    p["dve_after_cast"] = {0: 1, 1: 2}
    dve_vhc = {}
    p["dve_after_unit"] = {}
    for u, (kb, lo, hi) in enumerate(units):
        if dve_unit(u):
            dve += 1
            p["dve_after_unit"][u] = dve
        for j, at in VH_COPY_AT.items():
            if u == at:
                dve += 1
                dve_vhc[j] = dve
        if u == 35:  # kb12: PV(16) already on the PE queue
            dve += 1
            p["dve_after_ycopy_hi"] = dve
    dve += 1
    p["dve_after_ycopy_lo"] = dve
    p["dve_after_vhcopy"] = dve_vhc
    p["dve_total"] = dve

    # --- GP stream: one mask per kb, descending ---
    gp = 0
    p["gp_after_mask"] = {}
    for kb in range(NKB - 1, -1, -1):
        gp += 1
        p["gp_after_mask"][kb] = gp
    p["gp_total"] = gp
    return p


def build():
    plan = make_plan()
    units = plan["units"]
    last_unit = plan["last_unit"]

    nc = bass.Bass()

    d_qta = nc.declare_dram_parameter("qta", [65, QL], BF, isOutput=False)
    d_kta = nc.declare_dram_parameter("kta", [64, S], BF, isOutput=False)
    d_vta = nc.declare_dram_parameter("vta", [65, S], BF, isOutput=False)
    d_mt = nc.declare_dram_parameter("mt", [65, 64], BF, isOutput=False)
    d_nm = nc.declare_dram_parameter("nm", [65, 65], BF, isOutput=False)
    d_me = nc.declare_dram_parameter("me", [128, 128], BF, isOutput=False)
    d_mo = nc.declare_dram_parameter("mo", [128, 128], BF, isOutput=False)
    d_o = nc.declare_dram_parameter("o", [128, 1154], BF, isOutput=True)

    s_qta = nc.alloc_sbuf_tensor("s_qta", [65, QL], BF)
    s_kta = nc.alloc_sbuf_tensor("s_kta", [64, S], BF)
    s_vta = nc.alloc_sbuf_tensor("s_vta", [65, S], BF)
    s_mt = nc.alloc_sbuf_tensor("s_mt", [65, 64], BF)
    s_nm = nc.alloc_sbuf_tensor("s_nm", [65, 65], BF)
    s_me = nc.alloc_sbuf_tensor("s_me", [128, 128], BF)
    s_mo = nc.alloc_sbuf_tensor("s_mo", [128, 128], BF)
    s_G = nc.alloc_sbuf_tensor("s_G", [64, QL], BF)
    s_vhat = nc.alloc_sbuf_tensor("s_vhat", [128, NKB * VST], BF)
    s_P = [
        nc.alloc_sbuf_tensor("s_P0", [128, QL], BF),
        nc.alloc_sbuf_tensor("s_P1", [128, QL], BF),
        nc.alloc_sbuf_tensor("s_P2", [128, QL], BF),
    ]
    s_r = nc.alloc_sbuf_tensor("s_r", [128, 16], FP)
    s_Y = nc.alloc_sbuf_tensor("s_Y", [128, 1154], BF)

    with (
        nc.semaphore("sem_dq") as sem_dq,
        nc.semaphore("sem_dqh") as sem_dqh,
        nc.semaphore("sem_dk") as sem_dk,
        nc.semaphore("sem_dv") as sem_dv,
        nc.semaphore("sem_dkl") as sem_dkl,
        nc.semaphore("sem_dvl") as sem_dvl,
        nc.semaphore("sem_dm") as sem_dm,
        nc.semaphore("sem_pe") as sem_pe,
        nc.semaphore("sem_act") as sem_act,
        nc.semaphore("sem_dve") as sem_dve,
        nc.semaphore("sem_gp") as sem_gp,
        nc.semaphore("sem_out") as sem_out,
    ):
        with (
            nc.psum_tensor("p_strip", [128, 2048], FP) as p_strip,
            nc.psum_tensor("p_out", [128, 1536], FP) as p_out,
            nc.psum_tensor("p_vh", [128, 512], FP) as p_vh,
        ):
            with nc.Block() as blk:

                @blk.sync
                def _(sync):
                    sync.dma_start(s_mt[:, :], d_mt[:, :]).then_inc(sem_dq, 16)
                    sync.dma_start(s_kta[:, 3584:4096], d_kta[:, 3584:4096]).then_inc(
                        sem_dk, 16
                    )
                    sync.dma_start(s_nm[:, :], d_nm[:, :]).then_inc(sem_dv, 16)
                    sync.dma_start(s_vta[:, 3200:4096], d_vta[:, 3200:4096]).then_inc(
                        sem_dv, 16
                    )
                    sync.dma_start(s_kta[:, 2048:3584], d_kta[:, 2048:3584]).then_inc(
                        sem_dk, 16
                    )
                    sync.dma_start(s_me[:, :], d_me[:, :]).then_inc(sem_dm, 16)
                    sync.dma_start(s_mo[:, :], d_mo[:, :]).then_inc(sem_dm, 16)
                    sync.dma_start(s_kta[:, 0:2048], d_kta[:, 0:2048]).then_inc(
                        sem_dkl, 16
                    )
                    sync.dma_start(s_vta[:, 0:3200], d_vta[:, 0:3200]).then_inc(
                        sem_dvl, 16
                    )
                    sync.wait_ge(sem_dve, plan["dve_after_ycopy_hi"])
                    sync.dma_start(d_o[:, 577:1154], s_Y[0:128, 577:1154]).then_inc(
                        sem_out, 16
                    )
                    sync.wait_ge(sem_dve, plan["dve_after_ycopy_lo"])
                    sync.dma_start(d_o[:, 0:577], s_Y[0:128, 0:577]).then_inc(
                        sem_out, 16
                    )
                    sync.wait_ge(sem_out, 32)

                @blk.tensor
                def _(tensor):
                    def emit_vh_batch(j):
                        # vh-copy(j-1) completion is implied by the preceding
                        # scores-unit strip wait (always a later DVE position)
                        if j == 1:
                            tensor.wait_ge(sem_dvl, 16)
                        for i, kb in enumerate(VH_BATCHES[j]):
                            tensor.matmul(
                                p_vh[0:128, 65 * i : 65 * i + 65],
                                s_vta[0:65, 128 * kb : 128 * kb + 128],
                                s_nm[0:65, 0:65],
                                start=(i == 0),
                                stop=True,
                                skip_group_check=True,
                            ).then_inc(sem_pe, 1)

                    def emit_pv(kb):
                        tensor.wait_ge(sem_gp, plan["gp_after_mask"][kb])
                        for s in range(nslots(kb)):
                            tensor.matmul(
                                p_out[0:128, po_off(s) : po_off(s) + 65],
                                s_P[kb % 3][0:128, 128 * s : 128 * s + 128],
                                s_vhat[0:128, VST * kb : VST * kb + 65],
                                start=(kb == NKB - 1 and s % 7 == 0),
                                stop=(kb == 2 * s),
                                skip_group_check=True,
                            ).then_inc(sem_pe, 1)

                    # G projection into strip banks (cast out by DVE)
                    tensor.wait_ge(sem_dq, 16)
                    tensor.wait_ge(sem_dqh, 16)
                    for c0, c1 in chunks(0, 1024, 512):
                        tensor.matmul(
                            p_strip[0:64, c0:c1],
                            s_mt[0:65, 0:64],
                            s_qta[0:65, c0:c1],
                            start=True,
                            stop=True,
                        ).then_inc(sem_pe, 1)
                    tensor.wait_ge(sem_dqh, 32)
                    for c0, c1 in chunks(1024, QL, 512):
                        tensor.matmul(
                            p_strip[0:64, c0:c1],
                            s_mt[0:65, 0:64],
                            s_qta[0:65, c0:c1],
                            start=True,
                            stop=True,
                        ).then_inc(sem_pe, 1)
                    tensor.wait_ge(sem_dk, 16)
                    for u, (kb, lo, hi) in enumerate(units):
                        if kb == 27 and lo == 0:
                            tensor.wait_ge(sem_dk, 32)
                        if kb == 15 and lo == 0:
                            tensor.wait_ge(sem_dkl, 16)
                        if u < 2:
                            tensor.wait_ge(sem_dve, plan["dve_after_cast"][u])
                        if u >= 2:
                            if dve_unit(u - 2):
                                tensor.wait_ge(
                                    sem_dve, plan["dve_after_unit"][u - 2]
                                )
                            else:
                                tensor.wait_ge(
                                    sem_act, plan["act_after_unit"][u - 2]
                                )
                        base = 1024 * (u % 2)
                        for c0, c1 in chunks(lo, hi, 512):
                            tensor.matmul(
                                p_strip[0:128, base + c0 - lo : base + c1 - lo],
                                s_kta[0:64, 128 * kb : 128 * kb + 128],
                                s_G[0:64, c0:c1],
                                start=True,
                                stop=True,
                            ).then_inc(sem_pe, 1)
                        if u == last_unit[kb]:
                            if kb == NKB - 1:
                                tensor.wait_ge(sem_dv, 32)
                                emit_vh_batch(0)
                            for j, at in VH_EMIT_AT.items():
                                if kb == at:
                                    emit_vh_batch(j)
                            if kb < NKB - 2:
                                emit_pv(kb + 2)
                    emit_pv(1)
                    emit_pv(0)

                @blk.scalar
                def _(scalar):
                    scalar.dma_start(s_qta[:, 0:1024], d_qta[:, 0:1024]).then_inc(
                        sem_dqh, 16
                    )
                    scalar.dma_start(s_qta[:, 1024:2048], d_qta[:, 1024:2048]).then_inc(
                        sem_dqh, 16
                    )
                    # 1-elem exp right after mt lands preloads the act table
                    scalar.wait_ge(sem_dq, 16)
                    scalar.activation(
                        s_r[0:1, 0:1],
                        s_mt[0:1, 0:1],
                        mybir.ActivationFunctionType.Exp,
                    ).then_inc(sem_act, 1)

                    for u, (kb, lo, hi) in enumerate(units):
                        if dve_unit(u):
                            continue
                        scalar.wait_ge(sem_pe, plan["pe_after_sunit"][u])
                        base = 1024 * (u % 2)
                        scalar.activation(
                            s_P[kb % 3][0:128, lo:hi],
                            p_strip[0:128, base : base + hi - lo],
                            mybir.ActivationFunctionType.Exp,
                            scale=0.125,
                        ).then_inc(sem_act, 1)

                @blk.vector
                def _(vector):
                    def emit_vh_copy(j):
                        vector.wait_ge(sem_pe, plan["pe_after_vhb"][j])
                        n = len(VH_BATCHES[j])
                        kb_lo = VH_BATCHES[j][0]
                        src = p_vh[0:128, 0 : 65 * n].rearrange(
                            "p (n c) -> p n c", c=65
                        )
                        dst = s_vhat[0:128, VST * kb_lo : VST * (kb_lo + n)].rearrange(
                            "p (n c) -> p n c", c=VST
                        )[:, :, 0:65]
                        vector.tensor_copy(dst, src).then_inc(sem_dve, 1)

                    for ci in range(2):
                        vector.wait_ge(sem_pe, plan["pe_after_g"][ci])
                        vector.tensor_copy(
                            s_G[0:64, 1024 * ci : 1024 * ci + 1024],
                            p_strip[0:64, 1024 * ci : 1024 * ci + 1024],
                        ).then_inc(sem_dve, 1)
                    for u, (kb, lo, hi) in enumerate(units):
                        if dve_unit(u):
                            vector.wait_ge(sem_pe, plan["pe_after_sunit"][u])
                            base = 1024 * (u % 2)
                            vector.tensor_scalar(
                                s_P[kb % 3][0:128, lo:hi].bitcast(I16),
                                p_strip[0:128, base : base + hi - lo],
                                SCH_A,
                                SCH_B,
                                mybir.AluOpType.mult,
                                mybir.AluOpType.add,
                            ).then_inc(sem_dve, 1)
                        for j, at in VH_COPY_AT.items():
                            if u == at:
                                if j == 0:
                                    vector.wait_ge(sem_dv, 32)
                                emit_vh_copy(j)
                        if u == 35:
                            vector.wait_ge(sem_pe, plan["pe_after_pv"][16])
                            vector.tensor_copy(
                                s_Y[0:128, 577:1154], p_out[0:128, 577:1154]
                            ).then_inc(sem_dve, 1)
                    vector.wait_ge(sem_pe, plan["pe_after_pv"][0])
                    vector.tensor_copy(
                        s_Y[0:128, 0:577], p_out[0:128, 0:577]
                    ).then_inc(sem_dve, 1)

                @blk.gpsimd
                def _(gpsimd):
                    gpsimd.wait_ge(sem_dm, 32)
                    for kb in range(NKB - 1, -1, -1):
                        ub = bnd_unit(kb)
                        if dve_unit(ub):
                            gpsimd.wait_ge(sem_dve, plan["dve_after_unit"][ub])
                        else:
                            gpsimd.wait_ge(sem_act, plan["act_after_unit"][ub])
                        sb = min(kb // 2, NSLOT - 1)
                        m = s_me if kb % 2 == 0 else s_mo
                        gpsimd.tensor_mul(
                            s_P[kb % 3][0:128, 128 * sb : 128 * sb + 128],
                            s_P[kb % 3][0:128, 128 * sb : 128 * sb + 128],
                            m[0:128, 0:128],
                        ).then_inc(sem_gp, 1)

    return nc


_NC_CACHE = []


def _get_nc():
    if not _NC_CACHE:
        _NC_CACHE.append(build())
    return _NC_CACHE[0]


def _make_in_maps(q, k, v, Wq, bq, Wk, bk, Wv, bv, Wp, bp):
    f32 = np.float32
    bf16 = mybir.dt.np(BF)

    def prep(x):
        return np.ascontiguousarray(np.asarray(x, dtype=f32))

    q, k, v = prep(q), prep(k), prep(v)
    Wq, Wk, Wv, Wp = prep(Wq), prep(Wk), prep(Wv), prep(Wp)
    bq, bk, bv, bp = prep(bq), prep(bk), prep(bv), prep(bp)

    A = Wq.T @ Wk
    M64 = np.concatenate([A.T, (Wk.T @ bq)[:, None]], axis=1)  # [64, 65]
    mt = np.ascontiguousarray(M64.T).astype(bf16)  # [65, 64] lhsT

    Wpv = Wp @ Wv
    c = Wp @ bv + bp
    Nmat = np.zeros((65, 65), f32)
    Nmat[0:64, 0:64] = Wpv.T
    Nmat[64, 0:64] = c
    Nmat[64, 64] = 1.0
    nm = np.ascontiguousarray(Nmat).astype(bf16)

    ones1 = np.ones((1,), f32)
    tril = np.tril(np.ones((128, 128), f32)).astype(bf16)
    zeros = np.zeros((128, 128), bf16)
    ones_m = np.ones((128, 128), bf16)

    in_maps = []
    for cid in range(8):
        b, h = cid // 2, cid % 2
        qsel = q[b].reshape(32, 128, D)[h::2].reshape(QL, D)
        qta = np.concatenate([qsel.T, np.broadcast_to(ones1, (1, QL))], axis=0)
        kta = k[b].T
        vta = np.concatenate([v[b].T, np.broadcast_to(ones1, (1, S))], axis=0)
        in_maps.append(
            {
                "qta": np.ascontiguousarray(qta.astype(bf16)),
                "kta": np.ascontiguousarray(kta.astype(bf16)),
                "vta": np.ascontiguousarray(vta.astype(bf16)),
                "mt": mt,
                "nm": nm,
                "me": tril if h == 0 else zeros,
                "mo": ones_m if h == 0 else tril,
            }
        )
    return in_maps


def run(inputs, trace=False, **kw):
    nc = _get_nc()
    in_maps = _make_in_maps(**inputs)
    res = run_bass_kernel_spmd(nc, in_maps, core_ids=list(range(8)), trace=trace, **kw)
    out = np.empty((B, S, D), np.float32)
    for cid in range(8):
        b, h = cid // 2, cid % 2
        po = np.asarray(res.results[cid]["o"], dtype=np.float32)  # [128, 1536]
        acc = np.stack(
            [po[:, po_off(s) : po_off(s) + 64] for s in range(NSLOT)]
        )  # [16, 128, 64]
        l = np.stack([po[:, po_off(s) + 64] for s in range(NSLOT)])  # [16, 128]
        out[b].reshape(32, 128, D)[h::2] = acc / l[:, :, None]
    return out, res


def kernel(**inputs):
    out, _ = run(inputs, trace=False)
    return out


# revision 38
# speedup vs baseline: 1.0218x; 1.0218x over previous
"""Trainium2 Bass kernel: single-head attention with QKV+output projections.

Reference math (B=4, S=4096, D=64):
    Q = q@Wq.T+bq; K = k@Wk.T+bk; V = v@Wv.T+bv
    s = (Q @ K.T) / 8, masked -inf where i > j  (query i attends keys j >= i)
    out = softmax(s) @ V @ Wp.T + bp

Sharding (8 cores): core c -> batch b = c//2, parity h = c%2; 16 local
query slots of 128 rows (global tiles t = 2s+h), all 4096 keys.

Weight folding (host, O(D^3) only):
    S[i,j] = g_i . k_j + c_i ; the per-query c_i cancels in softmax, so
        g = M64 @ [q;1],  M64 = [A^T | Wk^T bq]  (64x65),  A = Wq^T Wk
        and scores use a pure 64-row k stationary (fast weight load).
    o_i = (sum_j p_ij vh_j) / l_i,  vh_j = [Wpv v_j + c; 1],
        Wpv = Wp Wv,  c = Wp bv + bp  (bp folds in since sum_j p_ij/l_i = 1;
        the ones column accumulates l alongside the output).

Device, per key block kb = 31..0 (descending; slot s finalizes at kb=2s):
    scores:  p_strip[128k, Nq(kb)] = kta_blk[64,128]^T @ G[64,Nq]
    exp:     SPLIT across two engines so the strip double-buffer drains
             ~2x faster and the PE stays dense:
               even units -> ACT exp (scale=1/8, bf16 out)
               odd units  -> DVE Schraudolph: i16=round(23.083*s+16248.59)
                 through an int16-bitcast AP; the bits ARE bf16 exp(s/8)
                 to ~2% rms (softmax renormalizes; l stays consistent
                 because it is summed from the same P values).
    mask:    GpSimd tril/zeros multiply on the boundary slot (sem_gp),
             keeping DVE free for exp work.
    PV flip: p_out[128q, 65] += P_tile[128,128]^T @ vhat_blk[128,65];
             the moving operand is the SMALL side, and ldweights pipeline
             under it (measured 54ns/slot = 65cyc at the fixed 1.2GHz).
    vhat     on-PE in batches (65-col matmuls from vta x folded N) through
             a 1-bank PSUM scratch; DVE copies it out.
    output:  NO on-device softmax divide - the raw accumulator (with l in
             column 64 of each slot) is copied to SBUF as bf16 in two
             waves (slots 8-15 after PV(16), 0-7 at the end) and DMA'd;
             the host divides by l. Kills the Ln/Exp/mult finale chain.

Prologue: DMA issue is spread over Sync+ACT queues; kta streams in three
chunks (kb 28-31 first) and qta in halves so G and the first score
blocks start as soon as their slice lands; a 1-elem exp right after mt
lands preloads the ACT table off the critical path.

HW notes: PE is pinned at the 1.2GHz p-state here (no HAM unthrottle,
never ramps to 2.4); matmul cost ~= moving columns x 0.833ns with
ldweights hidden, so the floor is ~47us of moving columns (scores 34816
+ PV 65x272 + vhat 2080 + G 2048) plus sem/bubble overhead. fp8
anywhere (even stationary-only) measurably slows the whole chip - keep
everything bf16. Measured ~77us (baseline 79us; 144us transposed
layout). Engine busy: PE ~64us (bound), ACT ~38, DVE ~35, GpSimd ~25.
"""

import numpy as np

import concourse.bass as bass
import concourse.mybir as mybir
from concourse.bass_utils import run_bass_kernel_spmd

B, S, D = 4, 4096, 64
NSLOT = 16
NKB = 32
QL = NSLOT * 128  # 2048
VST = 80  # vhat sbuf stride (65 used)

FP = mybir.dt.float32
BF = mybir.dt.bfloat16
I16 = mybir.dt.int16

# Schraudolph bf16-exp constants: bits(bf16 exp(s/8)) ~= 16*log2(e)*s +
# 128*(127 - sigma), sigma=0.0579 zero-mean
SCH_A = 23.083120654223414
SCH_B = 16248.5889

# vhat production batches (ascending kb within each so the batched DVE
# copy-out has positive strides); batch j>=1 is emitted on PE after the
# scores of VH_EMIT_AT[j], its copy goes after the DVE exp of VH_COPY_AT[j]
VH_BATCHES = [list(range(25, 32)), list(range(18, 25)),
              list(range(11, 18)), list(range(4, 11)),
              list(range(0, 4))]
VH_EMIT_AT = {1: 29, 2: 27, 3: 25, 4: 23}
VH_COPY_AT = {0: 1, 1: 3, 2: 7, 3: 11, 4: 15}  # after DVE exp of unit u


def dve_unit(u):
    return u % 2 == 1


def po_off(s):
    """p_out packing: 7 tiles per 512-f32 bank, stride 65."""
    return 512 * (s // 7) + 65 * (s % 7)


def nslots(kb):
    return min(kb // 2 + 1, NSLOT)


def bnd_unit(kb):
    """Unit index whose strip contains kb's boundary slot."""
    return 2 * (31 - kb) + 1 if kb >= 16 else 32 + (15 - kb)


def chunks(lo, hi, step):
    out = []
    c = lo
    while c < hi:
        out.append((c, min(c + step, hi)))
        c = out[-1][1]
    return out


def make_units():
    """Descending blocks, strip-chunked at 1024. Returns [(kb, lo, hi)]."""
    units = []
    for kb in range(NKB - 1, -1, -1):
        for lo, hi in chunks(0, 128 * nslots(kb), 1024):
            units.append((kb, lo, hi))
    return units


def tile_groups(s_lo, s_hi):
    """Split slot range at 7-tile PSUM bank-group boundaries."""
    out = []
    a = s_lo
    while a < s_hi:
        b = min(s_hi, (a // 7 + 1) * 7)
        out.append((a, b))
        a = b
    return out


def make_plan():
    units = make_units()
    last_unit = {}
    for u, (kb, lo, hi) in enumerate(units):
        last_unit[kb] = u

    p = {"units": units, "last_unit": last_unit}

    # --- PE stream ---
    pe = 0
    pe += 2  # G emitted as 4 chunks of 512 (PSUM bank limit)
    pe += 2
    p["pe_after_g"] = {0: 2, 1: 4}
    pe_vhb = {}
    p["pe_after_sunit"] = {}
    p["pe_after_pv"] = {}
    for u, (kb, lo, hi) in enumerate(units):
        pe += len(chunks(lo, hi, 512))
        p["pe_after_sunit"][u] = pe
        if u == last_unit[kb]:
            if kb == NKB - 1:
                pe += len(VH_BATCHES[0])
                pe_vhb[0] = pe
            for j, at in VH_EMIT_AT.items():
                if kb == at:
                    pe += len(VH_BATCHES[j])
                    pe_vhb[j] = pe
            if kb < NKB - 2:
                pe += nslots(kb + 2)
                p["pe_after_pv"][kb + 2] = pe
    pe += nslots(1)
    p["pe_after_pv"][1] = pe
    pe += nslots(0)
    p["pe_after_pv"][0] = pe
    p["pe_after_vhb"] = pe_vhb
    p["pe_total"] = pe

    # --- ACT stream: preload + even-unit exps (no finale: host divides) ---
    act = 1
    p["act_after_unit"] = {}
    for u, (kb, lo, hi) in enumerate(units):
        if dve_unit(u):
            continue
        act += 1
        p["act_after_unit"][u] = act
    p["act_total"] = act

    # --- DVE stream: casts + odd-unit exps + vh copies + finale mults ---
    dve = 0
    dve += 2  # G casts
    p["dve_after_cast"] = {0: 1, 1: 2}
    dve_vhc = {}
    p["dve_after_unit"] = {}
    for u, (kb, lo, hi) in enumerate(units):
        if dve_unit(u):
            dve += 1
            p["dve_after_unit"][u] = dve
        for j, at in VH_COPY_AT.items():
            if u == at:
                dve += 1
                dve_vhc[j] = dve
        if u == 35:  # kb12: PV(16) already on the PE queue
            dve += 1
            p["dve_after_ycopy_hi"] = dve
    dve += 1
    p["dve_after_ycopy_lo"] = dve
    p["dve_after_vhcopy"] = dve_vhc
    p["dve_total"] = dve

    # --- GP stream: one mask per kb, descending ---
    gp = 0
    p["gp_after_mask"] = {}
    for kb in range(NKB - 1, -1, -1):
        gp += 1
        p["gp_after_mask"][kb] = gp
    p["gp_total"] = gp
    return p


def build():
    plan = make_plan()
    units = plan["units"]
    last_unit = plan["last_unit"]

    nc = bass.Bass()

    d_qta = nc.declare_dram_parameter("qta", [65, QL], BF, isOutput=False)
    d_kta = nc.declare_dram_parameter("kta", [64, S], BF, isOutput=False)
    d_vta = nc.declare_dram_parameter("vta", [65, S], BF, isOutput=False)
    d_mt = nc.declare_dram_parameter("mt", [65, 64], BF, isOutput=False)
    d_nm = nc.declare_dram_parameter("nm", [65, 65], BF, isOutput=False)
    d_me = nc.declare_dram_parameter("me", [128, 128], BF, isOutput=False)
    d_mo = nc.declare_dram_parameter("mo", [128, 128], BF, isOutput=False)
    d_o = nc.declare_dram_parameter("o", [128, 1154], BF, isOutput=True)

    s_qta = nc.alloc_sbuf_tensor("s_qta", [65, QL], BF)
    s_kta = nc.alloc_sbuf_tensor("s_kta", [64, S], BF)
    s_vta = nc.alloc_sbuf_tensor("s_vta", [65, S], BF)
    s_mt = nc.alloc_sbuf_tensor("s_mt", [65, 64], BF)
    s_nm = nc.alloc_sbuf_tensor("s_nm", [65, 65], BF)
    s_me = nc.alloc_sbuf_tensor("s_me", [128, 128], BF)
    s_mo = nc.alloc_sbuf_tensor("s_mo", [128, 128], BF)
    s_G = nc.alloc_sbuf_tensor("s_G", [64, QL], BF)
    s_vhat = nc.alloc_sbuf_tensor("s_vhat", [128, NKB * VST], BF)
    s_P = [
        nc.alloc_sbuf_tensor("s_P0", [128, QL], BF),
        nc.alloc_sbuf_tensor("s_P1", [128, QL], BF),
        nc.alloc_sbuf_tensor("s_P2", [128, QL], BF),
    ]
    s_r = nc.alloc_sbuf_tensor("s_r", [128, 16], FP)
    s_Y = nc.alloc_sbuf_tensor("s_Y", [128, 1154], BF)

    with (
        nc.semaphore("sem_dq") as sem_dq,
        nc.semaphore("sem_dqh") as sem_dqh,
        nc.semaphore("sem_dk") as sem_dk,
        nc.semaphore("sem_dv") as sem_dv,
        nc.semaphore("sem_dkl") as sem_dkl,
        nc.semaphore("sem_dvl") as sem_dvl,
        nc.semaphore("sem_dm") as sem_dm,
        nc.semaphore("sem_pe") as sem_pe,
        nc.semaphore("sem_act") as sem_act,
        nc.semaphore("sem_dve") as sem_dve,
        nc.semaphore("sem_gp") as sem_gp,
        nc.semaphore("sem_out") as sem_out,
    ):
        with (
            nc.psum_tensor("p_strip", [128, 2048], FP) as p_strip,
            nc.psum_tensor("p_out", [128, 1536], FP) as p_out,
            nc.psum_tensor("p_vh", [128, 512], FP) as p_vh,
        ):
            with nc.Block() as blk:

                @blk.sync
                def _(sync):
                    sync.dma_start(s_mt[:, :], d_mt[:, :]).then_inc(sem_dq, 16)
                    sync.dma_start(s_kta[:, 3584:4096], d_kta[:, 3584:4096]).then_inc(
                        sem_dk, 16
                    )
                    sync.dma_start(s_qta[:, 1024:2048], d_qta[:, 1024:2048]).then_inc(
                        sem_dqh, 16
                    )
                    sync.dma_start(s_nm[:, :], d_nm[:, :]).then_inc(sem_dv, 16)
                    sync.dma_start(s_vta[:, 3200:4096], d_vta[:, 3200:4096]).then_inc(
                        sem_dv, 16
                    )
                    sync.dma_start(s_kta[:, 2048:3584], d_kta[:, 2048:3584]).then_inc(
                        sem_dk, 16
                    )
                    sync.dma_start(s_me[:, :], d_me[:, :]).then_inc(sem_dm, 16)
                    sync.dma_start(s_kta[:, 0:2048], d_kta[:, 0:2048]).then_inc(
                        sem_dkl, 16
                    )
                    sync.dma_start(s_vta[:, 0:3200], d_vta[:, 0:3200]).then_inc(
                        sem_dvl, 16
                    )
                    sync.wait_ge(sem_dve, plan["dve_after_ycopy_hi"])
                    sync.dma_start(d_o[:, 577:1154], s_Y[0:128, 577:1154]).then_inc(
                        sem_out, 16
                    )
                    sync.wait_ge(sem_dve, plan["dve_after_ycopy_lo"])
                    sync.dma_start(d_o[:, 0:577], s_Y[0:128, 0:577]).then_inc(
                        sem_out, 16
                    )
                    sync.wait_ge(sem_out, 32)

                @blk.tensor
                def _(tensor):
                    def emit_vh_batch(j):
                        # vh-copy(j-1) completion is implied by the preceding
                        # scores-unit strip wait (always a later DVE position)
                        if j == 1:
                            tensor.wait_ge(sem_dvl, 16)
                        for i, kb in enumerate(VH_BATCHES[j]):
                            tensor.matmul(
                                p_vh[0:128, 65 * i : 65 * i + 65],
                                s_vta[0:65, 128 * kb : 128 * kb + 128],
                                s_nm[0:65, 0:65],
                                start=(i == 0),
                                stop=True,
                                skip_group_check=True,
                            ).then_inc(sem_pe, 1)

                    def emit_pv(kb):
                        tensor.wait_ge(sem_gp, plan["gp_after_mask"][kb])
                        for s in range(nslots(kb)):
                            tensor.matmul(
                                p_out[0:128, po_off(s) : po_off(s) + 65],
                                s_P[kb % 3][0:128, 128 * s : 128 * s + 128],
                                s_vhat[0:128, VST * kb : VST * kb + 65],
                                start=(kb == NKB - 1 and s % 7 == 0),
                                stop=(kb == 2 * s),
                                skip_group_check=True,
                            ).then_inc(sem_pe, 1)

                    # G projection into strip banks (cast out by DVE)
                    tensor.wait_ge(sem_dq, 16)
                    tensor.wait_ge(sem_dqh, 16)
                    for c0, c1 in chunks(0, 1024, 512):
                        tensor.matmul(
                            p_strip[0:64, c0:c1],
                            s_mt[0:65, 0:64],
                            s_qta[0:65, c0:c1],
                            start=True,
                            stop=True,
                        ).then_inc(sem_pe, 1)
                    tensor.wait_ge(sem_dqh, 32)
                    for c0, c1 in chunks(1024, QL, 512):
                        tensor.matmul(
                            p_strip[0:64, c0:c1],
                            s_mt[0:65, 0:64],
                            s_qta[0:65, c0:c1],
                            start=True,
                            stop=True,
                        ).then_inc(sem_pe, 1)
                    tensor.wait_ge(sem_dk, 16)
                    for u, (kb, lo, hi) in enumerate(units):
                        if kb == 27 and lo == 0:
                            tensor.wait_ge(sem_dk, 32)
                        if kb == 15 and lo == 0:
                            tensor.wait_ge(sem_dkl, 16)
                        if u < 2:
                            tensor.wait_ge(sem_dve, plan["dve_after_cast"][u])
                        if u >= 2:
                            if dve_unit(u - 2):
                                tensor.wait_ge(
                                    sem_dve, plan["dve_after_unit"][u - 2]
                                )
                            else:
                                tensor.wait_ge(
                                    sem_act, plan["act_after_unit"][u - 2]
                                )
                        base = 1024 * (u % 2)
                        for c0, c1 in chunks(lo, hi, 512):
                            tensor.matmul(
                                p_strip[0:128, base + c0 - lo : base + c1 - lo],
                                s_kta[0:64, 128 * kb : 128 * kb + 128],
                                s_G[0:64, c0:c1],
                                start=True,
                                stop=True,
                            ).then_inc(sem_pe, 1)
                        if u == last_unit[kb]:
                            if kb == NKB - 1:
                                tensor.wait_ge(sem_dv, 32)
                                emit_vh_batch(0)
                            for j, at in VH_EMIT_AT.items():
                                if kb == at:
                                    emit_vh_batch(j)
                            if kb < NKB - 2:
                                emit_pv(kb + 2)
                    emit_pv(1)
                    emit_pv(0)

                @blk.scalar
                def _(scalar):
                    scalar.dma_start(s_qta[:, 0:1024], d_qta[:, 0:1024]).then_inc(
                        sem_dqh, 16
                    )
                    scalar.dma_start(s_mo[:, :], d_mo[:, :]).then_inc(sem_dm, 16)
                    # 1-elem exp right after mt lands preloads the act table
                    scalar.wait_ge(sem_dq, 16)
                    scalar.activation(
                        s_r[0:1, 0:1],
                        s_mt[0:1, 0:1],
                        mybir.ActivationFunctionType.Exp,
                    ).then_inc(sem_act, 1)

                    for u, (kb, lo, hi) in enumerate(units):
                        if dve_unit(u):
                            continue
                        scalar.wait_ge(sem_pe, plan["pe_after_sunit"][u])
                        base = 1024 * (u % 2)
                        scalar.activation(
                            s_P[kb % 3][0:128, lo:hi],
                            p_strip[0:128, base : base + hi - lo],
                            mybir.ActivationFunctionType.Exp,
                            scale=0.125,
                        ).then_inc(sem_act, 1)

                @blk.vector
                def _(vector):
                    def emit_vh_copy(j):
                        vector.wait_ge(sem_pe, plan["pe_after_vhb"][j])
                        n = len(VH_BATCHES[j])
                        kb_lo = VH_BATCHES[j][0]
                        src = p_vh[0:128, 0 : 65 * n].rearrange(
                            "p (n c) -> p n c", c=65
                        )
                        dst = s_vhat[0:128, VST * kb_lo : VST * (kb_lo + n)].rearrange(
                            "p (n c) -> p n c", c=VST
                        )[:, :, 0:65]
                        vector.tensor_copy(dst, src).then_inc(sem_dve, 1)

                    for ci in range(2):
                        vector.wait_ge(sem_pe, plan["pe_after_g"][ci])
                        vector.tensor_copy(
                            s_G[0:64, 1024 * ci : 1024 * ci + 1024],
                            p_strip[0:64, 1024 * ci : 1024 * ci + 1024],
                        ).then_inc(sem_dve, 1)
                    for u, (kb, lo, hi) in enumerate(units):
                        if dve_unit(u):
                            vector.wait_ge(sem_pe, plan["pe_after_sunit"][u])
                            base = 1024 * (u % 2)
                            vector.tensor_scalar(
                                s_P[kb % 3][0:128, lo:hi].bitcast(I16),
                                p_strip[0:128, base : base + hi - lo],
                                SCH_A,
                                SCH_B,
                                mybir.AluOpType.mult,
                                mybir.AluOpType.add,
                            ).then_inc(sem_dve, 1)
                        for j, at in VH_COPY_AT.items():
                            if u == at:
                                if j == 0:
                                    vector.wait_ge(sem_dv, 32)
                                emit_vh_copy(j)
                        if u == 35:
                            vector.wait_ge(sem_pe, plan["pe_after_pv"][16])
                            vector.tensor_copy(
                                s_Y[0:128, 577:1154], p_out[0:128, 577:1154]
                            ).then_inc(sem_dve, 1)
                    vector.wait_ge(sem_pe, plan["pe_after_pv"][0])
                    vector.tensor_copy(
                        s_Y[0:128, 0:577], p_out[0:128, 0:577]
                    ).then_inc(sem_dve, 1)

                @blk.gpsimd
                def _(gpsimd):
                    gpsimd.wait_ge(sem_dm, 32)
                    for kb in range(NKB - 1, -1, -1):
                        ub = bnd_unit(kb)
                        if dve_unit(ub):
                            gpsimd.wait_ge(sem_dve, plan["dve_after_unit"][ub])
                        else:
                            gpsimd.wait_ge(sem_act, plan["act_after_unit"][ub])
                        sb = min(kb // 2, NSLOT - 1)
                        m = s_me if kb % 2 == 0 else s_mo
                        gpsimd.tensor_mul(
                            s_P[kb % 3][0:128, 128 * sb : 128 * sb + 128],
                            s_P[kb % 3][0:128, 128 * sb : 128 * sb + 128],
                            m[0:128, 0:128],
                        ).then_inc(sem_gp, 1)

    return nc


_NC_CACHE = []


def _get_nc():
    if not _NC_CACHE:
        _NC_CACHE.append(build())
    return _NC_CACHE[0]


def _make_in_maps(q, k, v, Wq, bq, Wk, bk, Wv, bv, Wp, bp):
    f32 = np.float32
    bf16 = mybir.dt.np(BF)

    def prep(x):
        return np.ascontiguousarray(np.asarray(x, dtype=f32))

    q, k, v = prep(q), prep(k), prep(v)
    Wq, Wk, Wv, Wp = prep(Wq), prep(Wk), prep(Wv), prep(Wp)
    bq, bk, bv, bp = prep(bq), prep(bk), prep(bv), prep(bp)

    A = Wq.T @ Wk
    M64 = np.concatenate([A.T, (Wk.T @ bq)[:, None]], axis=1)  # [64, 65]
    mt = np.ascontiguousarray(M64.T).astype(bf16)  # [65, 64] lhsT

    Wpv = Wp @ Wv
    c = Wp @ bv + bp
    Nmat = np.zeros((65, 65), f32)
    Nmat[0:64, 0:64] = Wpv.T
    Nmat[64, 0:64] = c
    Nmat[64, 64] = 1.0
    nm = np.ascontiguousarray(Nmat).astype(bf16)

    ones1 = np.ones((1,), f32)
    tril = np.tril(np.ones((128, 128), f32)).astype(bf16)
    zeros = np.zeros((128, 128), bf16)
    ones_m = np.ones((128, 128), bf16)

    in_maps = []
    for cid in range(8):
        b, h = cid // 2, cid % 2
        qsel = q[b].reshape(32, 128, D)[h::2].reshape(QL, D)
        qta = np.concatenate([qsel.T, np.broadcast_to(ones1, (1, QL))], axis=0)
        kta = k[b].T
        vta = np.concatenate([v[b].T, np.broadcast_to(ones1, (1, S))], axis=0)
        in_maps.append(
            {
                "qta": np.ascontiguousarray(qta.astype(bf16)),
                "kta": np.ascontiguousarray(kta.astype(bf16)),
                "vta": np.ascontiguousarray(vta.astype(bf16)),
                "mt": mt,
                "nm": nm,
                "me": tril if h == 0 else zeros,
                "mo": ones_m if h == 0 else tril,
            }
        )
    return in_maps


def run(inputs, trace=False, **kw):
    nc = _get_nc()
    in_maps = _make_in_maps(**inputs)
    res = run_bass_kernel_spmd(nc, in_maps, core_ids=list(range(8)), trace=trace, **kw)
    out = np.empty((B, S, D), np.float32)
    for cid in range(8):
        b, h = cid // 2, cid % 2
        po = np.asarray(res.results[cid]["o"], dtype=np.float32)  # [128, 1536]
        acc = np.stack(
            [po[:, po_off(s) : po_off(s) + 64] for s in range(NSLOT)]
        )  # [16, 128, 64]
        l = np.stack([po[:, po_off(s) + 64] for s in range(NSLOT)])  # [16, 128]
        out[b].reshape(32, 128, D)[h::2] = acc / l[:, :, None]
    return out, res


def kernel(**inputs):
    out, _ = run(inputs, trace=False)
    return out
